# revision 1
# baseline (speedup 1.0000x reference)
"""AtomTransformer windowed-attention kernel, sharded SPMD across 8 trn2 NeuronCores.

Sharding: data-parallel over batch (4) x sequence-parallel over the window
axis (2 halves of 127 windows), with a 2-window halo per side so the 3
layers can be computed without any inter-core communication. Per-layer
weights are replicated. Each shard computes 66 windows per layer; the
owned (exactly-correct) windows are gathered on the host.
"""
import numpy as np
import jax
import jax.numpy as jnp

B, N, D, DP = 4, 8100, 128, 16
H, DH, WQ, WK, DEPTH = 4, 32, 64, 128, 3
NW = 127              # global window count
LW = 66               # windows computed per shard (own + halo)
TQ = LW * WQ          # 4224 q tokens per shard
TL = TQ + 64          # 4288 local tokens (32-token margin each side)
W0 = (0, 61)          # first global window per half
T0 = (-32, 3872)      # global token of local index 0 per half


def _ln(x, eps=1e-5):
    m = jnp.mean(x, axis=-1, keepdims=True)
    v = jnp.var(x, axis=-1, keepdims=True)
    return (x - m) * jax.lax.rsqrt(v + eps)


def _adaln(x, s_ln, Wg, bg, Wb):
    return jax.nn.sigmoid(s_ln @ Wg + bg) * _ln(x) + s_ln @ Wb


def _shard_fn(xl, cl, pair, layers):
    # xl, cl: [TL, D] local repr/cond tokens; pair: [LW, WQ, WK, DP]
    idx = jnp.arange(LW)[:, None] * WQ + jnp.arange(WK)[None, :]
    c_ln = _ln(cl)
    sq = c_ln[32:32 + TQ].reshape(LW, WQ, D)
    sk = c_ln[idx]
    pair_ln = _ln(pair)
    inv = np.float32(1.0 / np.sqrt(DH))
    x = xl
    for p in layers:
        qx = x[32:32 + TQ].reshape(LW, WQ, D)
        ctx = x[idx]
        aq = _adaln(qx, sq, p["a_Wg"], p["a_bg"], p["a_Wb"])
        ak = _adaln(ctx, sk, p["a_Wg"], p["a_bg"], p["a_Wb"])
        q = (aq @ p["Wq"] + p["bq"]).reshape(LW, WQ, H, DH)
        k = (ak @ p["Wk"]).reshape(LW, WK, H, DH)
        v = (ak @ p["Wv"]).reshape(LW, WK, H, DH)
        bias = pair_ln @ p["Wb_pair"]  # [LW, WQ, WK, H]
        logits = jnp.einsum("nqhd,nkhd->nhqk", q, k) * inv \
            + jnp.einsum("nqkh->nhqk", bias)
        att = jax.nn.softmax(logits, axis=-1)
        o = jnp.einsum("nhqk,nkhd->nqhd", att, v)
        g = jax.nn.sigmoid((aq @ p["Wgate"]).reshape(o.shape))
        o = (g * o).reshape(LW, WQ, H * DH) @ p["Wo"]
        attn_out = jax.nn.sigmoid(sq @ p["Wcg"] + p["bcg"]) * o
        af = _adaln(qx, sq, p["f_Wg"], p["f_bg"], p["f_Wb"])
        h = jax.nn.silu(af @ p["W1"]) * (af @ p["W2"])
        ff = jax.nn.sigmoid(sq @ p["f_Wcg"] + p["f_bcg"]) * (h @ p["Wout"])
        xw = (qx + attn_out + ff).reshape(TQ, D)
        x = x.at[32:32 + TQ].set(xw)
    return x


def _slice_tokens(full, t0):
    # full: [N, D] -> [TL, D] local tokens with zero margins outside [0, N)
    out = np.zeros((TL, D), np.float32)
    lo, hi = max(t0, 0), min(t0 + TL, N)
    out[lo - t0:hi - t0] = full[lo:hi]
    return out


def kernel(single_repr, single_cond, pair_cond, params):
    single_repr = np.asarray(single_repr, np.float32)
    single_cond = np.asarray(single_cond, np.float32)
    pair_cond = np.asarray(pair_cond, np.float32)
    layers = [
        {k: np.asarray(v, np.float32) for k, v in p.items()}
        for p in params["layers"]
    ]

    devices = jax.devices()[:8]
    fn = jax.jit(_shard_fn)

    futures = []
    for c, dev in enumerate(devices):
        b, hf = c // 2, c % 2
        xl = jax.device_put(_slice_tokens(single_repr[b], T0[hf]), dev)
        cl = jax.device_put(_slice_tokens(single_cond[b], T0[hf]), dev)
        pr = jax.device_put(pair_cond[b, W0[hf]:W0[hf] + LW], dev)
        ly = jax.device_put(layers, dev)
        futures.append(fn(xl, cl, pr, ly))

    out = np.empty((B, N, D), np.float32)
    for c, res in enumerate(futures):
        b, hf = c // 2, c % 2
        res = np.asarray(res)
        if hf == 0:
            out[b, 0:4096] = res[32:32 + 4096]       # windows [0, 64)
        else:
            out[b, 4096:N] = res[224:224 + (N - 4096)]  # windows [64, 127)
    return out


# revision 3
# speedup vs baseline: 1.5878x; 1.5878x over previous
"""AtomTransformer windowed-attention kernel, sharded SPMD across 8 trn2 NeuronCores.

Sharding: data-parallel over batch (4) x sequence-parallel over the window
axis (2 halves of 127 windows), with a 2-window halo per side so the 3
layers can be computed without any inter-core communication. Per-layer
weights are replicated. Each shard computes 66 windows per layer; the
owned (exactly-correct) windows are gathered on the host.
"""
import numpy as np
import jax
import jax.numpy as jnp

B, N, D, DP = 4, 8100, 128, 16
H, DH, WQ, WK, DEPTH = 4, 32, 64, 128, 3
NW = 127              # global window count
LW = 66               # windows computed per shard (own + halo)
TQ = LW * WQ          # 4224 q tokens per shard
TL = TQ + 64          # 4288 local tokens (32-token margin each side)
W0 = (0, 61)          # first global window per half
T0 = (-32, 3872)      # global token of local index 0 per half


def _ln(x, eps=1e-5):
    m = jnp.mean(x, axis=-1, keepdims=True)
    v = jnp.var(x, axis=-1, keepdims=True)
    return (x - m) * jax.lax.rsqrt(v + eps)


def _adaln(x, s_ln, Wg, bg, Wb):
    return jax.nn.sigmoid(s_ln @ Wg + bg) * _ln(x) + s_ln @ Wb


def _shard_fn(xl, cl, pair, layers):
    # xl, cl: [TL, D] local repr/cond tokens; pair: [LW, WQ, WK, DP] (f16)
    pair = pair.astype(jnp.float32)
    idx = jnp.arange(LW)[:, None] * WQ + jnp.arange(WK)[None, :]
    c_ln = _ln(cl)
    sq = c_ln[32:32 + TQ].reshape(LW, WQ, D)
    sk = c_ln[idx]
    pair_ln = _ln(pair)
    inv = np.float32(1.0 / np.sqrt(DH))
    x = xl
    for p in layers:
        qx = x[32:32 + TQ].reshape(LW, WQ, D)
        ctx = x[idx]
        aq = _adaln(qx, sq, p["a_Wg"], p["a_bg"], p["a_Wb"])
        ak = _adaln(ctx, sk, p["a_Wg"], p["a_bg"], p["a_Wb"])
        q = (aq @ p["Wq"] + p["bq"]).reshape(LW, WQ, H, DH)
        k = (ak @ p["Wk"]).reshape(LW, WK, H, DH)
        v = (ak @ p["Wv"]).reshape(LW, WK, H, DH)
        bias = pair_ln @ p["Wb_pair"]  # [LW, WQ, WK, H]
        logits = jnp.einsum("nqhd,nkhd->nhqk", q, k) * inv \
            + jnp.einsum("nqkh->nhqk", bias)
        att = jax.nn.softmax(logits, axis=-1)
        o = jnp.einsum("nhqk,nkhd->nqhd", att, v)
        g = jax.nn.sigmoid((aq @ p["Wgate"]).reshape(o.shape))
        o = (g * o).reshape(LW, WQ, H * DH) @ p["Wo"]
        attn_out = jax.nn.sigmoid(sq @ p["Wcg"] + p["bcg"]) * o
        af = _adaln(qx, sq, p["f_Wg"], p["f_bg"], p["f_Wb"])
        h = jax.nn.silu(af @ p["W1"]) * (af @ p["W2"])
        ff = jax.nn.sigmoid(sq @ p["f_Wcg"] + p["f_bcg"]) * (h @ p["Wout"])
        xw = (qx + attn_out + ff).reshape(TQ, D)
        x = x.at[32:32 + TQ].set(xw)
    return x


def _slice_tokens(full, t0):
    # full: [N, D] -> [TL, D] local tokens with zero margins outside [0, N)
    out = np.zeros((TL, D), np.float32)
    lo, hi = max(t0, 0), min(t0 + TL, N)
    out[lo - t0:hi - t0] = full[lo:hi]
    return out


def kernel(single_repr, single_cond, pair_cond, params):
    single_repr = np.asarray(single_repr, np.float32)
    single_cond = np.asarray(single_cond, np.float32)
    pair_cond = np.asarray(pair_cond, np.float32)
    layers = [
        {k: np.asarray(v, np.float32) for k, v in p.items()}
        for p in params["layers"]
    ]

    devices = jax.devices()[:8]
    fn = jax.jit(_shard_fn)

    shards = []
    for c, dev in enumerate(devices):
        b, hf = c // 2, c % 2
        xl = jax.device_put(_slice_tokens(single_repr[b], T0[hf]), dev)
        cl = jax.device_put(_slice_tokens(single_cond[b], T0[hf]), dev)
        pr = jax.device_put(
            pair_cond[b, W0[hf]:W0[hf] + LW].astype(np.float16), dev)
        ly = jax.device_put(layers, dev)
        shards.append((xl, cl, pr, ly))
    futures = [fn(*s) for s in shards]

    out = np.empty((B, N, D), np.float32)
    for c, res in enumerate(futures):
        b, hf = c // 2, c % 2
        res = np.asarray(res)
        if hf == 0:
            out[b, 0:4096] = res[32:32 + 4096]       # windows [0, 64)
        else:
            out[b, 4096:N] = res[224:224 + (N - 4096)]  # windows [64, 127)
    return out
